# revision 19
# baseline (speedup 1.0000x reference)
"""Trainium2 Bass kernel: masked-LSTM readout over to_dense_batch'd graphs.

v2 strategy (8 NeuronCores, SPMD single program):
 - Host: per-graph lengths from sorted `index`; graphs globally sorted by
   length (desc) and dealt round-robin to 8 cores, so all cores share one
   step schedule N_t. Host densifies x into a block-major padded tensor per
   core (fp16), feature-major [64, rows].
 - Device per step: rhs = [x_t ; H_{t-1}] stacked on 128 partitions (H==2h
   written into the x tile's bottom half by the previous step), so each
   gate-pair needs ONE matmul with contract 128. Two independent column
   pieces pipeline the serial chain. Gates: sigmoid ACT for (f,i); tanh ACT
   with per-partition scale (0.5;1) for (o,g) giving o~=2sig(o)-1 and
   g~=tanh(g) directly. Cell update in fp16 on DVE: 3 tensor_tensor (2x
   mode) + 1 scalar_tensor_tensor; tanh(c) on ACT. Final h snapshot via
   predicated copy at each graph's last valid step.
 - Host: gather per-core outputs (H=2h -> h), invert the deal permutation.
"""

import numpy as np

MAXLEN = 100
B = 8192
NCORES = 8
G = B // NCORES          # graph columns per core
H = 64
F = 64
TW = 20                  # steps per time block
CHUNK = 512              # psum bank width (f32 cols)

_CACHE = {}


def _build_and_compile(schedule, weights):
    import concourse.bacc as bacc
    import concourse.mybir as mybir
    from concourse import tile

    N_t, blocks, snap, MW = schedule
    wfi_np, wog_np, bfi_np, bog_np, scog_np = weights
    fp16 = mybir.dt.float16
    f32 = mybir.dt.float32
    T_end = len(N_t)
    ROWS_TOT = sum(Wb * nst for (_, nst, Wb, _) in blocks)
    XT_W = max(Wb * nst for (_, nst, Wb, _) in blocks)

    nc = bacc.Bacc("TRN2", target_bir_lowering=False)
    xd_d = nc.dram_tensor("xd", [64, ROWS_TOT], fp16, kind="ExternalInput")
    msk_d = nc.dram_tensor("msk", [64, max(MW, 1)], mybir.dt.uint8, kind="ExternalInput")
    out_d = nc.dram_tensor("outh", [64, G], fp16, kind="ExternalOutput")
    # piece0 gate order (f,i)/(o,g); piece1 swapped (i,f)/(g,o) so that its
    # f-gate and o~ sit on partitions 64:128 matching the packed state half
    wfi_d = nc.dram_tensor("wfi", [128, 128], fp16, kind="ExternalInput")
    wog_d = nc.dram_tensor("wog", [128, 128], fp16, kind="ExternalInput")
    wfi1_d = nc.dram_tensor("wfi1", [128, 128], fp16, kind="ExternalInput")
    wog1_d = nc.dram_tensor("wog1", [128, 128], fp16, kind="ExternalInput")
    bfi_d = nc.dram_tensor("bfi", [128, 1], f32, kind="ExternalInput")
    bog_d = nc.dram_tensor("bog", [128, 1], f32, kind="ExternalInput")
    bfi1_d = nc.dram_tensor("bfi1", [128, 1], f32, kind="ExternalInput")
    bog1_d = nc.dram_tensor("bog1", [128, 1], f32, kind="ExternalInput")
    scog_d = nc.dram_tensor("scog", [128, 1], f32, kind="ExternalInput")
    scog1_d = nc.dram_tensor("scog1", [128, 1], f32, kind="ExternalInput")

    Sig = mybir.ActivationFunctionType.Sigmoid
    Tanh = mybir.ActivationFunctionType.Tanh
    Mult = mybir.AluOpType.mult
    Add = mybir.AluOpType.add

    # block index for each step, and column base within the block's tile
    blk_of = {}
    for bi, (t0, nst, Wb, row0) in enumerate(blocks):
        for ts in range(nst):
            blk_of[t0 + ts] = (bi, ts)

    with tile.TileContext(nc) as tc:
        with tc.tile_pool(name="state", bufs=1) as sp, \
             tc.tile_pool(name="xblk", bufs=2) as xp, \
             tc.tile_pool(name="psum", bufs=2, space="PSUM") as pp, \
             tc.tile_pool(name="gates", bufs=2) as gp:
            wfi = sp.tile([128, 128], fp16)
            nc.sync.dma_start(out=wfi, in_=wfi_d.ap())
            wog = sp.tile([128, 128], fp16)
            nc.sync.dma_start(out=wog, in_=wog_d.ap())
            wfi1 = sp.tile([128, 128], fp16)
            nc.sync.dma_start(out=wfi1, in_=wfi1_d.ap())
            wog1 = sp.tile([128, 128], fp16)
            nc.sync.dma_start(out=wog1, in_=wog1_d.ap())
            bfi = sp.tile([128, 1], f32)
            nc.sync.dma_start(out=bfi, in_=bfi_d.ap())
            bog = sp.tile([128, 1], f32)
            nc.sync.dma_start(out=bog, in_=bog_d.ap())
            bfi1 = sp.tile([128, 1], f32)
            nc.sync.dma_start(out=bfi1, in_=bfi1_d.ap())
            bog1 = sp.tile([128, 1], f32)
            nc.sync.dma_start(out=bog1, in_=bog1_d.ap())
            scog = sp.tile([128, 1], f32)
            nc.sync.dma_start(out=scog, in_=scog_d.ap())
            scog1 = sp.tile([128, 1], f32)
            nc.sync.dma_start(out=scog1, in_=scog1_d.ap())
            mskt = sp.tile([64, max(MW, 1)], mybir.dt.uint8)
            nc.sync.dma_start(out=mskt, in_=msk_d.ap())

            # c/tcc packed: piece0 (cols 0:512) on partitions 0:64,
            # piece1 (cols 512:1024, shifted by -512) on partitions 64:128
            c = sp.tile([128, 512], fp16, name="c")
            t1 = sp.tile([64, 1024], fp16, name="t1")
            t2 = sp.tile([64, 1024], fp16, name="t2")
            tcc = sp.tile([128, 512], fp16, name="tcc")
            hs = sp.tile([64, 1024], fp16, name="hs")
            outh = sp.tile([64, 1024], fp16, name="outh")
            nc.vector.memset(c[:, :], 0.0)
            nc.vector.memset(outh[:, :], 0.0)
            nc.vector.memset(hs[:, :], 0.0)

            def cslice(tile_, lo, hi):
                """Packed-state slice for absolute cols [lo,hi)."""
                if hi <= 512:
                    return tile_[0:64, lo:hi]
                assert lo >= 512
                return tile_[64:128, lo - 512:hi - 512]

            xts = {}  # live xt tiles by block index

            def make_xt(bi2):
                if bi2 in xts or bi2 >= len(blocks):
                    return
                _, nst2, Wb2, row02 = blocks[bi2]
                rows2 = Wb2 * nst2
                xt2 = xp.tile([128, XT_W], fp16, tag="xt", name=f"xt{bi2}")
                xts[bi2] = xt2
                # x occupies partitions 64:128; H (=2h) occupies 0:64
                nc.sync.dma_start(out=xt2[64:128, 0:rows2],
                                  in_=xd_d.ap()[:, row02:row02 + rows2])

            for bi, (t0, nsteps, Wb, row0) in enumerate(blocks):
                make_xt(bi)
                make_xt(bi + 1)  # prefetch next block's x
                xt = xts[bi]
                if bi == 0:
                    nc.vector.memset(xt[0:64, 0:Wb], 0.0)

                for ts in range(nsteps):
                    t = t0 + ts
                    n = N_t[t]
                    if n == 0:
                        continue
                    base = ts * Wb
                    # independent column pieces (each <= CHUNK wide)
                    if n > CHUNK:
                        pieces = [(0, CHUNK), (CHUNK, n)]
                    elif n > 32:
                        m = (n // 2 + 15) & ~15
                        pieces = [(0, m), (m, n)]
                    else:
                        pieces = [(0, n)]

                    # next-step destination for H
                    if t + 1 < T_end:
                        nbi, nts = blk_of[t + 1]
                        Wb_n = blocks[nbi][2]
                        base_n = nts * Wb_n
                        wA = min(n, Wb_n)
                        xt_n = xts[nbi]
                    else:
                        wA = 0
                        xt_n = None

                    sgf, pss = {}, {}
                    sws = [lo >= CHUNK for (lo, hi) in pieces]
                    for pi, (lo, hi) in enumerate(pieces):
                        w = hi - lo
                        ps = pp.tile([128, 1024], mybir.dt.float32,
                                     tag=f"ps{pi}", name=f"ps{pi}")
                        pss[pi] = ps
                        nc.tensor.matmul(out=ps[:, 0:w],
                                         lhsT=(wfi1 if sws[pi] else wfi)[:, :],
                                         rhs=xt[:, base + lo:base + hi],
                                         start=True, stop=True)
                        nc.tensor.matmul(out=ps[:, CHUNK:CHUNK + w],
                                         lhsT=(wog1 if sws[pi] else wog)[:, :],
                                         rhs=xt[:, base + lo:base + hi],
                                         start=True, stop=True)
                    for pi, (lo, hi) in enumerate(pieces):
                        w = hi - lo
                        ps = pss[pi]
                        sg = gp.tile([128, 1024], fp16, tag=f"sg{pi}", name=f"sg{pi}")
                        sgf[pi] = sg
                        nc.scalar.activation(out=sg[:, 0:w], in_=ps[:, 0:w],
                                             func=Sig,
                                             bias=(bfi1 if sws[pi] else bfi)[:, :])
                        nc.scalar.activation(out=sg[:, CHUNK:CHUNK + w],
                                             in_=ps[:, CHUNK:CHUNK + w],
                                             func=Tanh,
                                             bias=(bog1 if sws[pi] else bog)[:, :],
                                             scale=(scog1 if sws[pi] else scog)[:, :])
                    # t1 = sig(f) * c ; t2 = sig(i) * tanh(g) ; c = t1 + t2
                    # (interleave pieces so neither chain clogs the DVE queue)
                    for pi, (lo, hi) in enumerate(pieces):
                        w = hi - lo
                        fr = (64, 128) if sws[pi] else (0, 64)  # f-gate rows
                        nc.vector.tensor_tensor(
                            out=t1[:, lo:hi], in0=sgf[pi][fr[0]:fr[1], 0:w],
                            in1=cslice(c, lo, hi), op=Mult)
                    for pi, (lo, hi) in enumerate(pieces):
                        w = hi - lo
                        ir = (0, 64) if sws[pi] else (64, 128)  # i-gate rows
                        nc.vector.tensor_tensor(
                            out=t2[:, lo:hi], in0=sgf[pi][ir[0]:ir[1], 0:w],
                            in1=sgf[pi][ir[0]:ir[1], CHUNK:CHUNK + w], op=Mult)
                    for pi, (lo, hi) in enumerate(pieces):
                        nc.vector.tensor_tensor(
                            out=cslice(c, lo, hi), in0=t1[:, lo:hi],
                            in1=t2[:, lo:hi], op=Add)
                    # one packed tanh covers both pieces when n > 512
                    if n > CHUNK:
                        nc.scalar.activation(out=tcc[:, 0:CHUNK],
                                             in_=c[:, 0:CHUNK], func=Tanh)
                    else:
                        nc.scalar.activation(out=tcc[0:64, 0:n],
                                             in_=c[0:64, 0:n], func=Tanh)
                    # H = (o~ + 1) * tanh(c)  == 2h
                    for pi, (lo, hi) in enumerate(pieces):
                        sg = sgf[pi]
                        orow = (64, 128) if sws[pi] else (0, 64)  # o~ rows
                        for (a, b_) in _split((lo, hi), wA):
                            wseg = b_ - a
                            if wseg <= 0:
                                continue
                            if b_ <= wA:
                                dst = xt_n[0:64, base_n + a:base_n + b_]
                            else:
                                dst = hs[:, a:b_]
                            nc.vector.scalar_tensor_tensor(
                                out=dst,
                                in0=sg[orow[0]:orow[1], CHUNK + a - lo:CHUNK + b_ - lo],
                                scalar=1.0, in1=cslice(tcc, a, b_), op0=Add, op1=Mult)
                    # snapshot graphs ending at step t
                    for (lo, hi, moff) in snap[t]:
                        for (a, b_) in _split((lo, hi), wA):
                            wseg = b_ - a
                            if wseg <= 0:
                                continue
                            if b_ <= wA:
                                src = xt_n[0:64, base_n + a:base_n + b_]
                            else:
                                src = hs[:, a:b_]
                            nc.vector.copy_predicated(
                                out=outh[:, a:b_],
                                mask=mskt[:, moff + a - lo:moff + b_ - lo],
                                data=src)
            nc.sync.dma_start(out=out_d.ap()[:, 0:G], in_=outh[:, 0:G])
    nc.compile()
    return nc


def _split(rng, cut):
    """Split [lo,hi) at cut into segments lying fully below or above cut."""
    lo, hi = rng
    if cut <= lo:
        return [(lo, hi)]
    if cut >= hi:
        return [(lo, hi)]
    return [(lo, cut), (cut, hi)]


def _plan(lens):
    """Global schedule from capped lengths [B]."""
    order = np.argsort(-lens, kind="stable")
    lens_sorted = lens[order]
    T_end = int(lens_sorted.max())
    len_c = lens_sorted.reshape(G, NCORES).T            # [NCORES, G]
    t_ax = np.arange(T_end + 1)
    n_c = (len_c[:, :, None] > t_ax[None, None, :]).sum(axis=1)
    N_t = n_c.max(axis=0)                               # [T_end+1], N_t[T_end]==0
    blocks = []
    row0 = 0
    t0 = 0
    while t0 < T_end:
        nsteps = min(TW, T_end - t0)
        Wb = max(16, int(np.ceil(N_t[t0] / 16) * 16))
        blocks.append((t0, nsteps, Wb, row0))
        row0 += Wb * nsteps
        t0 += nsteps
    snap = []
    moff = 0
    mask_cols = []
    for t in range(T_end):
        nt1 = n_c[:, t + 1]
        lo = int(nt1.min())
        hi = int(n_c[:, t].max())
        pieces = []
        if hi > lo:
            m = np.zeros((NCORES, hi - lo), np.uint8)
            for cc in range(NCORES):
                a, b_ = int(nt1[cc]), int(n_c[cc, t])
                m[cc, max(a - lo, 0):max(b_ - lo, 0)] = 1
            mask_cols.append(m)
            pieces.append((lo, hi, moff))
            moff += hi - lo
        snap.append(pieces)
    masks = (np.concatenate(mask_cols, axis=1) if mask_cols
             else np.zeros((NCORES, 1), np.uint8))
    return order, len_c, n_c, [int(x) for x in N_t[:T_end]], blocks, snap, masks


LAST_RUN = {}


def _install_ntff_shim():
    import sys, types
    if "antenv.axon_hooks" in sys.modules:
        return
    try:
        from trn_agent_boot.trn_boot import _ntff_profile_via_ctypes
        hook = _ntff_profile_via_ctypes("/opt/axon/libaxon_pjrt.so")
    except Exception:
        hook = None
    m = types.ModuleType("antenv.axon_hooks")
    m._hook = hook
    m.get_axon_ntff_profile_hook = lambda: m._hook
    m.set_axon_ntff_profile_hook = lambda h: setattr(m, "_hook", h)
    sys.modules["antenv.axon_hooks"] = m


def kernel(x, W_ih, W_hh, b_ih, b_hh, index, dim_size, _trace=False):
    from concourse.bass_utils import run_bass_kernel_spmd
    if _trace:
        import concourse.bass_utils as _bu
        _install_ntff_shim()
        _bu.upload_artifacts = lambda d: d

    x = np.asarray(x)
    index = np.asarray(index).astype(np.int64)
    W_ih = np.asarray(W_ih, dtype=np.float32)
    W_hh = np.asarray(W_hh, dtype=np.float32)
    b_ih = np.asarray(b_ih, dtype=np.float32)
    b_hh = np.asarray(b_hh, dtype=np.float32)

    assert int(dim_size) == B, f"kernel hardcodes B={B}, got dim_size={int(dim_size)}"
    counts = np.bincount(index, minlength=B).astype(np.int64)
    offsets = np.concatenate([[0], np.cumsum(counts)[:-1]])
    lens = np.minimum(counts, MAXLEN)

    order, len_c, n_c, N_t, blocks, snap, masks = _plan(lens)

    # --- weights (torch gate order i,f,g,o) ---
    b = (b_ih + b_hh).reshape(4, H)
    Wi, Wf, Wg, Wo = W_ih.reshape(4, H, F)
    Ui, Uf, Ug, Uo = W_hh.reshape(4, H, H)
    # rhs rows 0:64 carry H = 2h (h-weights pre-halved); rows 64:128 carry x.
    def stack2(A, Bm):
        return np.concatenate(
            [np.concatenate([0.5 * A[0].T, 0.5 * Bm[0].T], 1),
             np.concatenate([A[1].T, Bm[1].T], 1)], 0).astype(np.float16)
    wfi = stack2((Uf, Wf), (Ui, Wi))       # (f,i) gate order
    wog = stack2((Uo, Wo), (Ug, Wg))       # (o,g)
    wfi1 = stack2((Ui, Wi), (Uf, Wf))      # swapped (i,f) for piece1
    wog1 = stack2((Ug, Wg), (Uo, Wo))      # swapped (g,o)
    # og ACT: tanh(pre*scale + bias); o-rows scale .5 (o~=2sig(o)-1), g scale 1
    bfi = np.concatenate([b[1], b[0]]).reshape(128, 1).astype(np.float32)
    bog = np.concatenate([0.5 * b[3], b[2]]).reshape(128, 1).astype(np.float32)
    scog = np.concatenate([0.5 * np.ones(64), np.ones(64)]).reshape(128, 1).astype(np.float32)
    bfi1 = np.concatenate([b[0], b[1]]).reshape(128, 1).astype(np.float32)
    bog1 = np.concatenate([b[2], 0.5 * b[3]]).reshape(128, 1).astype(np.float32)
    scog1 = np.concatenate([np.ones(64), 0.5 * np.ones(64)]).reshape(128, 1).astype(np.float32)

    # --- per-core dense input, feature-major [64, rows] ---
    x16 = x.astype(np.float16)
    in_maps = []
    for cN in range(NCORES):
        gids = order[np.arange(G) * NCORES + cN]
        lens_cj = len_c[cN]
        offs_cj = offsets[gids]
        parts = []
        for (t0, nsteps, Wb, row0) in blocks:
            tsl = np.arange(t0, t0 + nsteps)
            node = offs_cj[:Wb, None] + tsl[None, :]             # [Wb, nsteps]
            valid = tsl[None, :] < lens_cj[:Wb, None]
            node = np.clip(node, 0, x.shape[0] - 1)
            blk = np.where(valid[:, :, None], x16[node], np.float16(0))
            # row r = ts*Wb + col
            parts.append(blk.transpose(1, 0, 2).reshape(nsteps * Wb, 64))
        xd = np.ascontiguousarray(np.concatenate(parts, axis=0).T)
        msk = np.ascontiguousarray(
            np.broadcast_to(masks[cN][None, :], (64, masks.shape[1])))
        in_maps.append({"xd": xd, "msk": msk, "wfi": wfi, "wog": wog,
                        "wfi1": wfi1, "wog1": wog1, "bfi": bfi, "bog": bog,
                        "bfi1": bfi1, "bog1": bog1, "scog": scog,
                        "scog1": scog1})

    import hashlib
    key = hashlib.sha1(
        (repr((N_t, blocks, snap)) ).encode()
        + W_ih.tobytes() + W_hh.tobytes() + b_ih.tobytes() + b_hh.tobytes()
    ).hexdigest()
    if key not in _CACHE:
        _CACHE[key] = _build_and_compile(
            (N_t, blocks, snap, masks.shape[1]),
            (wfi, wog, bfi, bog, scog))
    nc = _CACHE[key]

    res = run_bass_kernel_spmd(nc, in_maps, core_ids=list(range(NCORES)),
                               trace=_trace)
    LAST_RUN["res"] = res

    out = np.zeros((B, H), np.float32)
    for cN in range(NCORES):
        hT = res.results[cN]["outh"].astype(np.float32)  # [64, G] == 2h
        gids = order[np.arange(G) * NCORES + cN]
        out[gids] = 0.5 * hT.T
    return out
